# revision 2
# baseline (speedup 1.0000x reference)
"""Causal single-head attention on 8 Trainium2 NeuronCores.

Problem: B=8, S=2048, E=768, HEAD=128, fp32.
  Xm = X * padding_mask[:, :, None]
  q/k/v = Xm @ W_{q,k,v}.T          [B, S, H]
  scores = (q @ k.T) / sqrt(H)  (causal)
  out = softmax(scores) @ v          [B, S, H]

Sharding: pure data-parallel over batch - core b computes batch b; the
tiny projection weights are replicated to every core.

v3 design notes (on top of the bf16 v2 baseline):
  - Startup: the first q-block's inputs stream in as per-eo-pair chunks
    spread over three HWDGE queues (xt0 chunks on the sync queue, w3
    chunks + consts on gpsimd, xt2/xt3 bulk on scalar) with per-chunk
    completion semaphores, so the first projection matmul issues as
    soon as (w3[eo01], xt0[eo01]) land (~2us after the preamble) rather
    than after whole-tensor DMAs. Warmup shrinks from 22 to 3 matmuls.
  - Projection matmuls for block b+1 are interleaved as small work
    items into the attention pair-pipeline of block b, filling the PE
    bubbles that previously appeared while waiting on ACT exp. Block
    3's projection is split: qT/kT items run inside attn(2), vT +
    transpose items inside attn(3)'s early pairs (legal: pair g only
    touches k-tiles 2g,2g+1, so block-3 k/v tiles are needed only from
    pair 6 on).
  - Per-block qT/kT/vT/v SBUF tiles so interleaved projection writes
    can never alias attention reads of earlier blocks.
  - outT is drained in bf16 (host divides in fp32) in 256-column
    halves as soon as their accumulation completes, on rotating DMA
    queues; den goes out as one 8KB DMA at the end. This cuts the
    post-last-matmul drain from ~4.4us to <1us.
"""

import math
import sys

import numpy as np

sys.path.insert(0, "/opt/trn_rl_repo")

import ml_dtypes

B, S, E, H = 8, 2048, 768, 128
EO = E // 128          # 6 e-chunks
NJB = S // 512         # 4 q-blocks of 512
SCALE = float(1.0 / math.sqrt(H))

_CACHE = {}


def _emit_body(nc, tc, pools, dram):
    import concourse.bass as bass  # noqa: F401
    from concourse import mybir

    f32 = mybir.dt.float32
    bf16 = mybir.dt.bfloat16
    Exp = mybir.ActivationFunctionType.Exp

    singles, prb_p, ps_proj, ps_sc, ps_o, ps_d = pools
    (xt_d, w3_d, consts_d, outT_d, den_d) = dram

    sb = _CACHE["sb"]
    if not sb:
        for jb in range(NJB):
            sb[f"xt{jb}"] = singles.tile(
                [128, EO, 512], bf16, tag=f"xt{jb}", name=f"xt{jb}"
            )
            for nm in ("qT", "kT", "vT", "v"):
                sb[f"{nm}{jb}"] = singles.tile(
                    [128, 512], bf16, tag=f"{nm}{jb}", name=f"{nm}{jb}"
                )
            sb[f"outF{jb}"] = singles.tile(
                [128, 512], bf16, tag=f"outF{jb}", name=f"outF{jb}"
            )
        sb["w3"] = singles.tile([128, EO, 3, H], bf16, tag="w3", name="w3")
        sb["consts"] = singles.tile([128, 3, 128], bf16, tag="consts", name="consts")
        sb["denF"] = singles.tile([1, S], f32, tag="denF", name="denF")
        sb["warm"] = singles.tile([128, 512], bf16, tag="warm", name="warm")

    xt_ap = xt_d.ap()
    w3_ap = w3_d.ap()
    outT_ap = outT_d.ap()
    den_ap = den_d.ap()

    # ---- prologue loads --------------------------------------------------
    # Startup-critical pieces go first on each queue, in chunks, so the
    # first projection matmuls can begin while the rest streams in.
    # warm-tile memset first so the PE warmup isn't gated on it.
    nc.vector.memset(sb["warm"], 0.125)
    # sync queue: xt0 in 3 eo-pair chunks (2KB/partition lines), then xt1.
    for p in range(3):
        nc.sync.dma_start(out=sb["xt0"][:, 2 * p : 2 * p + 2], in_=xt_ap[:, 0, 2 * p : 2 * p + 2])
    nc.sync.dma_start(out=sb["xt1"], in_=xt_ap[:, 1])
    # gpsimd queue: w3 in 3 eo-pair chunks, then consts.
    for p in range(3):
        nc.gpsimd.dma_start(out=sb["w3"][:, 2 * p : 2 * p + 2], in_=w3_ap[:, 2 * p : 2 * p + 2])
    nc.gpsimd.dma_start(out=sb["consts"], in_=consts_d.ap())
    # scalar queue (slow first-byte): later blocks, bulk.
    nc.scalar.dma_start(out=sb["xt2"], in_=xt_ap[:, 2])
    nc.scalar.dma_start(out=sb["xt3"], in_=xt_ap[:, 3])

    ident = sb["consts"][:, 0, :]
    triA = sb["consts"][:, 1, :]
    ones1 = sb["consts"][:, 2, 0:1]

    # Short PE warmup bridging the ~1.5us until the first chunks land;
    # primes the HAM activity window so real matmuls warm up sooner.
    ps_warm = ps_proj.tile([128, 512], f32, tag="proj", name="ps_warm")
    for _ in range(3):
        nc.tensor.matmul(
            ps_warm, lhsT=sb["warm"][:, 0:128], rhs=sb["warm"], start=True, stop=True
        )

    # ---- projection work items ------------------------------------------
    # proj(b) = 11 small PE items: 3 per weight (one per eo-pair, last one
    # adds the PSUM->SBUF copy) + 2 transpose items for v.
    def proj_items(b, parts=(0, 1, 2)):
        items = []
        cell = {}

        def mm_item(wi, p, nm, b=b):
            def run():
                if p == 0:
                    cell[wi] = ps_proj.tile(
                        [128, 512], f32, tag="proj", name=f"ps_{nm}{b}"
                    )
                ps = cell[wi]
                for eo in (2 * p, 2 * p + 1):
                    nc.tensor.matmul(
                        ps,
                        lhsT=sb["w3"][:, eo, wi, :],
                        rhs=sb[f"xt{b}"][:, eo, :],
                        start=(eo == 0),
                        stop=(eo == EO - 1),
                    )
                if p == 2:
                    nc.vector.tensor_copy(sb[f"{nm}{b}"], ps)

            return run

        def tr_item(half, b=b):
            def run():
                if half == 0:
                    cell["psv"] = ps_proj.tile(
                        [128, 512], bf16, tag="proj", name=f"psv{b}"
                    )
                psv = cell["psv"]
                for c in (2 * half, 2 * half + 1):
                    nc.tensor.transpose(
                        psv[:, 128 * c : 128 * (c + 1)],
                        sb[f"vT{b}"][:, 128 * c : 128 * (c + 1)],
                        ident,
                    )
                if half == 1:
                    nc.vector.tensor_copy(sb[f"v{b}"], psv)

            return run

        for wi, nm in ((0, "qT"), (1, "kT"), (2, "vT")):
            if wi not in parts:
                continue
            for p in range(3):
                items.append(mm_item(wi, p, nm))
        if 2 in parts:
            items.append(tr_item(0))
            items.append(tr_item(1))
        return items

    DRAIN_ENGS = (nc.gpsimd, nc.sync, nc.scalar)

    # ---- attention for q-block b, interleaving `items` into the pairs ----
    def attn(b, items):
        nkt = 4 * (b + 1)          # causal: k tiles 0 .. 4b+3
        npr = nkt // 2
        pso = ps_o.tile([128, 512], f32, tag="o", name=f"pso_{b}")
        psd = ps_d.tile([1, 512], f32, tag="d", name=f"psd_{b}")
        qT = sb[f"qT{b}"]

        def off_of(i):
            return 128 * (i - 4 * b) if i >= 4 * b else 0

        def kt_of(i):
            return sb[f"kT{i // 4}"][:, 128 * (i % 4) : 128 * (i % 4 + 1)]

        def v_of(i):
            return sb[f"v{i // 4}"][:, 128 * (i % 4) : 128 * (i % 4 + 1)]

        def emit_scores(g):
            pssc = ps_sc.tile([128, 2, 512], f32, tag="sc", name=f"sc_{b}_{g}")
            for t in range(2):
                i = 2 * g + t
                diag = i >= 4 * b
                off = off_of(i)
                nc.tensor.matmul(
                    pssc[:, t, off:],
                    lhsT=kt_of(i),
                    rhs=qT[:, off:],
                    start=True,
                    stop=not diag,
                )
                if diag:  # add -400 strictly-upper triangle (k > q) pre-exp
                    nc.tensor.matmul(
                        pssc[:, t, off : off + 128],
                        lhsT=triA,
                        rhs=ident,
                        start=False,
                        stop=True,
                    )
            moff = off_of(2 * g)
            prb = prb_p.tile([128, 2, 512], bf16, tag="pr", name=f"prb_{b}_{g}")
            nc.scalar.activation(
                prb[:, :, moff:], pssc[:, :, moff:], Exp, scale=SCALE
            )
            return (g, prb)

        def emit_outden(pend, last):
            g, pprb = pend
            for t in range(2):
                i = 2 * g + t
                off = off_of(i)
                nc.tensor.matmul(
                    pso[:, off:],
                    lhsT=v_of(i),
                    rhs=pprb[:, t, off:],
                    start=(i == 0),
                    stop=last and t == 1,
                )
            for t in range(2):
                i = 2 * g + t
                off = off_of(i)
                nc.tensor.matmul(
                    psd[:, off:],
                    lhsT=ones1,
                    rhs=pprb[:, t, off:],
                    start=(i == 0),
                    stop=last and t == 1,
                )
            # cols [0:256] final once the off=128 diag tile has run: drain
            # early so the tail copy+DMA overlaps the last pair.
            if g == npr - 2:
                nc.vector.tensor_copy(sb[f"outF{b}"][:, 0:256], pso[:, 0:256])
                DRAIN_ENGS[b % 3].dma_start(
                    out=outT_ap[:, 512 * b : 512 * b + 256],
                    in_=sb[f"outF{b}"][:, 0:256],
                )

        # spread items over the early pairs (all before pair npr-2 when
        # possible, so block-3's own k/v items land before they're read).
        spread = max(1, min(npr - 1, 6))
        pipe = []
        for g in range(npr):
            pipe.append(emit_scores(g))
            if g < spread and items:
                budget = (len(items) + (spread - g) - 1) // (spread - g)
                for _ in range(budget):
                    if items:
                        items.pop(0)()
            if len(pipe) > 1:
                emit_outden(pipe.pop(0), last=False)
        while items:  # leftovers (small blocks)
            items.pop(0)()
        while pipe:
            p = pipe.pop(0)
            emit_outden(p, last=not pipe)

        # tail drain: cols [256:512] + this block's denominators
        nc.vector.tensor_copy(sb[f"outF{b}"][:, 256:512], pso[:, 256:])
        DRAIN_ENGS[(b + 1) % 3].dma_start(
            out=outT_ap[:, 512 * b + 256 : 512 * (b + 1)],
            in_=sb[f"outF{b}"][:, 256:512],
        )
        nc.vector.tensor_copy(sb["denF"][0:1, 512 * b : 512 * (b + 1)], psd)

    # ---- schedule --------------------------------------------------------
    for it in proj_items(0):
        it()
    attn(0, proj_items(1))
    attn(1, proj_items(2))
    attn(2, proj_items(3, parts=(0, 1)))       # qT/kT of block 3
    attn(3, proj_items(3, parts=(2,)))         # vT + transposes of block 3
    nc.sync.dma_start(out=den_ap, in_=sb["denF"])


def _build(repeat=1):
    key = ("nc", repeat)
    if key in _CACHE:
        return _CACHE[key]

    import concourse.tile as tile
    from concourse import bacc, mybir

    f32 = mybir.dt.float32
    bf16 = mybir.dt.bfloat16
    nc = bacc.Bacc("TRN2", target_bir_lowering=False, debug=False)

    xt_d = nc.dram_tensor("xt", [128, NJB, EO, 512], bf16, kind="ExternalInput")
    w3_d = nc.dram_tensor("w3", [128, EO, 3, H], bf16, kind="ExternalInput")
    consts_d = nc.dram_tensor("consts", [128, 3, 128], bf16, kind="ExternalInput")
    outT_d = nc.dram_tensor("outT", [128, S], bf16, kind="ExternalOutput")
    den_d = nc.dram_tensor("den", [1, S], f32, kind="ExternalOutput")
    dram = (xt_d, w3_d, consts_d, outT_d, den_d)

    _CACHE["sb"] = {}
    with tile.TileContext(nc) as tc:
        with (
            tc.tile_pool(name="singles", bufs=1) as singles,
            tc.tile_pool(name="probs", bufs=6) as prb_p,
            tc.tile_pool(name="ps_proj", bufs=2, space="PSUM") as ps_proj,
            tc.tile_pool(name="ps_sc", bufs=2, space="PSUM") as ps_sc,
            tc.tile_pool(name="ps_o", bufs=1, space="PSUM") as ps_o,
            tc.tile_pool(name="ps_d", bufs=1, space="PSUM") as ps_d,
        ):
            pools = (singles, prb_p, ps_proj, ps_sc, ps_o, ps_d)
            for _ in range(repeat):
                _emit_body(nc, tc, pools, dram)

    nc.compile()
    _CACHE[key] = nc
    return nc


def _prep_in_maps(X, padding_mask, W_q, W_k, W_v):
    X = np.asarray(X, dtype=np.float32)
    padding_mask = np.asarray(padding_mask, dtype=np.float32)

    def wprep(W):
        # [H, E] -> [E, H] -> [128(ei), EO, H] with ei innermost of E
        return np.asarray(W, dtype=np.float32).T.reshape(EO, 128, H).transpose(1, 0, 2)

    # [128, EO, 3, H]
    w3 = np.ascontiguousarray(
        np.stack([wprep(W_q), wprep(W_k), wprep(W_v)], axis=2)
    ).astype(ml_dtypes.bfloat16)
    ident = np.eye(128, dtype=np.float32)
    triA = -400.0 * np.triu(np.ones((128, 128), dtype=np.float32), 1)
    ones = np.ones((128, 128), dtype=np.float32)
    consts = np.ascontiguousarray(np.stack([ident, triA, ones], axis=1)).astype(
        ml_dtypes.bfloat16
    )  # [128, 3, 128]
    in_maps = []
    for b in range(B):
        Xm = X[b] * padding_mask[b][:, None]  # exact fp32 mask, then quantize
        in_maps.append(
            {
                "xt": np.ascontiguousarray(
                    # [S, E] -> [E, S] -> [128(ei), NJB, EO, 512]
                    Xm.T.reshape(EO, 128, NJB, 512).transpose(1, 2, 0, 3)
                ).astype(ml_dtypes.bfloat16),
                "w3": w3,
                "consts": consts,
            }
        )
    return in_maps


def _finish(res):
    # device wrote outT [128(h), S] bf16 and den [1, S]; out[q, h] = outT.T / den
    return (res["outT"].astype(np.float32).T / res["den"][0][:, None]).astype(
        np.float32
    )


def kernel(X, padding_mask, W_q, W_k, W_v):
    from concourse import bass2jax

    nc = _build(repeat=1)
    in_maps = _prep_in_maps(X, padding_mask, W_q, W_k, W_v)
    results = bass2jax.run_bass_via_pjrt(nc, in_maps, n_cores=B)
    return np.stack([_finish(results[b]) for b in range(B)], axis=0)


# revision 9
# speedup vs baseline: 1.2140x; 1.2140x over previous
"""Causal single-head attention on 8 Trainium2 NeuronCores.

Problem: B=8, S=2048, E=768, HEAD=128, fp32.
  Xm = X * padding_mask[:, :, None]
  q/k/v = Xm @ W_{q,k,v}.T          [B, S, H]
  scores = (q @ k.T) / sqrt(H)  (causal)
  out = softmax(scores) @ v          [B, S, H]

Sharding: pure data-parallel over batch - core b computes batch b; the
tiny projection weights are replicated to every core.

v3 design notes (on top of the bf16 v2 baseline):
  - Startup: the first q-block's inputs stream in as per-eo-pair chunks
    spread over three HWDGE queues (xt0 chunks on the sync queue, w3
    chunks + consts on gpsimd, xt2/xt3 bulk on scalar) with per-chunk
    completion semaphores, so the first projection matmul issues as
    soon as (w3[eo01], xt0[eo01]) land (~2us after the preamble) rather
    than after whole-tensor DMAs. Warmup shrinks from 22 to 3 matmuls.
  - Projection matmuls for block b+1 are interleaved as small work
    items into the attention pair-pipeline of block b, filling the PE
    bubbles that previously appeared while waiting on ACT exp. Block
    3's projection is split: qT/kT items run inside attn(2), vT +
    transpose items inside attn(3)'s early pairs (legal: pair g only
    touches k-tiles 2g,2g+1, so block-3 k/v tiles are needed only from
    pair 6 on).
  - Per-block qT/kT/vT/v SBUF tiles so interleaved projection writes
    can never alias attention reads of earlier blocks.
  - outT is drained in bf16 (host divides in fp32) in 256-column
    halves as soon as their accumulation completes, on rotating DMA
    queues; den goes out as one 8KB DMA at the end. This cuts the
    post-last-matmul drain from ~4.4us to <1us.
"""

import math
import sys

import numpy as np

sys.path.insert(0, "/opt/trn_rl_repo")

import ml_dtypes

B, S, E, H = 8, 2048, 768, 128
EO = E // 128          # 6 e-chunks
NJB = S // 512         # 4 q-blocks of 512
SCALE = float(1.0 / math.sqrt(H))

_CACHE = {}


def _emit_body(nc, tc, pools, dram):
    import concourse.bass as bass  # noqa: F401
    from concourse import mybir

    f32 = mybir.dt.float32
    bf16 = mybir.dt.bfloat16
    Exp = mybir.ActivationFunctionType.Exp

    singles, prb_p, ps_proj, ps_sc, ps_o, ps_d = pools
    (xt_d, w3_d, consts_d, outT_d, den_d) = dram

    sb = _CACHE["sb"]
    if not sb:
        for jb in range(NJB):
            sb[f"xt{jb}"] = singles.tile(
                [128, EO, 512], bf16, tag=f"xt{jb}", name=f"xt{jb}"
            )
            for nm in ("qT", "kT", "vT", "v"):
                sb[f"{nm}{jb}"] = singles.tile(
                    [128, 512], bf16, tag=f"{nm}{jb}", name=f"{nm}{jb}"
                )
            sb[f"outF{jb}"] = singles.tile(
                [128, 512], bf16, tag=f"outF{jb}", name=f"outF{jb}"
            )
        sb["w3"] = singles.tile([128, EO, 3, H], bf16, tag="w3", name="w3")
        sb["consts"] = singles.tile([128, 3, 128], bf16, tag="consts", name="consts")
        sb["denF"] = singles.tile([1, S], f32, tag="denF", name="denF")
        sb["warm"] = singles.tile([128, 512], bf16, tag="warm", name="warm")

    xt_ap = xt_d.ap()
    w3_ap = w3_d.ap()
    outT_ap = outT_d.ap()
    den_ap = den_d.ap()

    # ---- prologue loads --------------------------------------------------
    # Transfers are ordered globally by first use. The sync queue serves
    # within ~1.5us of kernel start and sustains ~250GB/s, so it carries
    # the critical sequence: w3[eo01] (first projection weights), then
    # xt0 in 3 eo-pair chunks (2KB/partition lines, per-chunk completion
    # semaphores so each projection matmul waits only on its own chunk),
    # then xt1. Secondary queues join ~2-4us later: scalar gets the rest
    # of w3 + consts + xt2/xt3 bulk. gpsimd stays free for output drains
    # (a queue with bulk input queued ahead would stall the tail DMAs).
    # warm-tile memset on gpsimd (its engine is live ~1us before vector).
    nc.gpsimd.memset(sb["warm"], 0.125)
    nc.sync.dma_start(out=sb["w3"][:, 0:2], in_=w3_ap[:, 0:2])
    for p in range(3):
        nc.sync.dma_start(
            out=sb["xt0"][:, 2 * p : 2 * p + 2], in_=xt_ap[:, 0, 2 * p : 2 * p + 2]
        )
    nc.sync.dma_start(out=sb["xt1"], in_=xt_ap[:, 1])
    nc.scalar.dma_start(out=sb["w3"][:, 2:4], in_=w3_ap[:, 2:4])
    nc.scalar.dma_start(out=sb["w3"][:, 4:6], in_=w3_ap[:, 4:6])
    nc.scalar.dma_start(out=sb["consts"], in_=consts_d.ap())
    nc.scalar.dma_start(out=sb["xt2"], in_=xt_ap[:, 2])
    nc.scalar.dma_start(out=sb["xt3"], in_=xt_ap[:, 3])

    ident = sb["consts"][:, 0, :]
    triA = sb["consts"][:, 1, :]
    ones1 = sb["consts"][:, 2, 0:1]

    # Short PE warmup bridging the ~2us until the first chunks land;
    # primes the HAM activity window so real matmuls warm up sooner.
    ps_warm = ps_proj.tile([128, 512], f32, tag="proj", name="ps_warm")
    for _ in range(5):
        nc.tensor.matmul(
            ps_warm, lhsT=sb["warm"][:, 0:128], rhs=sb["warm"], start=True, stop=True
        )

    # ---- projection work items ------------------------------------------
    # proj(b) = 11 small PE items: 3 per weight (one per eo-pair, last one
    # adds the PSUM->SBUF copy) + 2 transpose items for v.
    def proj_items(b, parts=(0, 1, 2)):
        items = []
        cell = {}

        def mm_item(wi, p, nm, b=b):
            def run():
                if p == 0:
                    cell[wi] = ps_proj.tile(
                        [128, 512], f32, tag="proj", name=f"ps_{nm}{b}"
                    )
                ps = cell[wi]
                for eo in (2 * p, 2 * p + 1):
                    nc.tensor.matmul(
                        ps,
                        lhsT=sb["w3"][:, eo, wi, :],
                        rhs=sb[f"xt{b}"][:, eo, :],
                        start=(eo == 0),
                        stop=(eo == EO - 1),
                    )
                if p == 2:
                    nc.vector.tensor_copy(sb[f"{nm}{b}"], ps)

            return run

        def tr_item(half, b=b):
            def run():
                if half == 0:
                    cell["psv"] = ps_proj.tile(
                        [128, 512], bf16, tag="proj", name=f"psv{b}"
                    )
                psv = cell["psv"]
                for c in (2 * half, 2 * half + 1):
                    nc.tensor.transpose(
                        psv[:, 128 * c : 128 * (c + 1)],
                        sb[f"vT{b}"][:, 128 * c : 128 * (c + 1)],
                        ident,
                    )
                if half == 1:
                    nc.vector.tensor_copy(sb[f"v{b}"], psv)

            return run

        for wi, nm in ((0, "qT"), (1, "kT"), (2, "vT")):
            if wi not in parts:
                continue
            for p in range(3):
                items.append(mm_item(wi, p, nm))
        if 2 in parts:
            items.append(tr_item(0))
            items.append(tr_item(1))
        return items

    # ---- attention for q-block b, interleaving `items` into the pairs ----
    def attn(b, items):
        nkt = 4 * (b + 1)          # causal: k tiles 0 .. 4b+3
        npr = nkt // 2
        pso = ps_o.tile([128, 512], f32, tag="o", name=f"pso_{b}")
        psd = ps_d.tile([1, 512], f32, tag="d", name=f"psd_{b}")
        qT = sb[f"qT{b}"]

        def off_of(i):
            return 128 * (i - 4 * b) if i >= 4 * b else 0

        def kt_of(i):
            return sb[f"kT{i // 4}"][:, 128 * (i % 4) : 128 * (i % 4 + 1)]

        def v_of(i):
            return sb[f"v{i // 4}"][:, 128 * (i % 4) : 128 * (i % 4 + 1)]

        def emit_scores(g):
            pssc = ps_sc.tile([128, 2, 512], f32, tag="sc", name=f"sc_{b}_{g}")
            for t in range(2):
                i = 2 * g + t
                diag = i >= 4 * b
                off = off_of(i)
                nc.tensor.matmul(
                    pssc[:, t, off:],
                    lhsT=kt_of(i),
                    rhs=qT[:, off:],
                    start=True,
                    stop=not diag,
                )
                if diag:  # add -400 strictly-upper triangle (k > q) pre-exp
                    nc.tensor.matmul(
                        pssc[:, t, off : off + 128],
                        lhsT=triA,
                        rhs=ident,
                        start=False,
                        stop=True,
                    )
            moff = off_of(2 * g)
            prb = prb_p.tile([128, 2, 512], bf16, tag="pr", name=f"prb_{b}_{g}")
            nc.scalar.activation(
                prb[:, :, moff:], pssc[:, :, moff:], Exp, scale=SCALE
            )
            return (g, prb)

        def emit_outden(pend, last):
            g, pprb = pend
            for t in range(2):
                i = 2 * g + t
                off = off_of(i)
                nc.tensor.matmul(
                    pso[:, off:],
                    lhsT=v_of(i),
                    rhs=pprb[:, t, off:],
                    start=(i == 0),
                    stop=last and t == 1,
                )
            for t in range(2):
                i = 2 * g + t
                off = off_of(i)
                nc.tensor.matmul(
                    psd[:, off:],
                    lhsT=ones1,
                    rhs=pprb[:, t, off:],
                    start=(i == 0),
                    stop=last and t == 1,
                )
            # cols [0:256] final once the off=128 diag tile has run: drain
            # early so the tail copy+DMA overlaps the last pair. Only the
            # last block DMAs its halves separately (tail latency); the
            # others go out as one 1KB-line transfer on the idle gpsimd
            # queue after the tail copy.
            if g == npr - 2:
                nc.vector.tensor_copy(sb[f"outF{b}"][:, 0:256], pso[:, 0:256])
                if b == NJB - 1:
                    nc.scalar.dma_start(
                        out=outT_ap[:, 512 * b : 512 * b + 256],
                        in_=sb[f"outF{b}"][:, 0:256],
                    )

        # spread items over the early pairs (all before pair npr-2 when
        # possible, so block-3's own k/v items land before they're read).
        spread = max(1, min(npr - 1, 6))
        pipe = []
        for g in range(npr):
            pipe.append(emit_scores(g))
            if g < spread and items:
                budget = (len(items) + (spread - g) - 1) // (spread - g)
                for _ in range(budget):
                    if items:
                        items.pop(0)()
            if len(pipe) > 1:
                emit_outden(pipe.pop(0), last=False)
        while items:  # leftovers (small blocks)
            items.pop(0)()
        while pipe:
            p = pipe.pop(0)
            emit_outden(p, last=not pipe)

        # tail drain: cols [256:512] + this block's denominators
        nc.vector.tensor_copy(sb[f"outF{b}"][:, 256:512], pso[:, 256:])
        if b == NJB - 1:
            nc.sync.dma_start(
                out=outT_ap[:, 512 * b + 256 : 512 * (b + 1)],
                in_=sb[f"outF{b}"][:, 256:512],
            )
        else:
            nc.gpsimd.dma_start(
                out=outT_ap[:, 512 * b : 512 * (b + 1)], in_=sb[f"outF{b}"]
            )
        nc.vector.tensor_copy(sb["denF"][0:1, 512 * b : 512 * (b + 1)], psd)

    # ---- schedule --------------------------------------------------------
    for it in proj_items(0):
        it()
    attn(0, proj_items(1))
    attn(1, proj_items(2))
    attn(2, proj_items(3, parts=(0, 1)))       # qT/kT of block 3
    attn(3, proj_items(3, parts=(2,)))         # vT + transposes of block 3
    nc.scalar.dma_start(out=den_ap, in_=sb["denF"])


def _build(repeat=1):
    key = ("nc", repeat)
    if key in _CACHE:
        return _CACHE[key]

    import concourse.tile as tile
    from concourse import bacc, mybir

    f32 = mybir.dt.float32
    bf16 = mybir.dt.bfloat16
    nc = bacc.Bacc("TRN2", target_bir_lowering=False, debug=False)

    xt_d = nc.dram_tensor("xt", [128, NJB, EO, 512], bf16, kind="ExternalInput")
    w3_d = nc.dram_tensor("w3", [128, EO, 3, H], bf16, kind="ExternalInput")
    consts_d = nc.dram_tensor("consts", [128, 3, 128], bf16, kind="ExternalInput")
    outT_d = nc.dram_tensor("outT", [128, S], bf16, kind="ExternalOutput")
    den_d = nc.dram_tensor("den", [1, S], f32, kind="ExternalOutput")
    dram = (xt_d, w3_d, consts_d, outT_d, den_d)

    _CACHE["sb"] = {}
    with tile.TileContext(nc) as tc:
        with (
            tc.tile_pool(name="singles", bufs=1) as singles,
            tc.tile_pool(name="probs", bufs=6) as prb_p,
            tc.tile_pool(name="ps_proj", bufs=2, space="PSUM") as ps_proj,
            tc.tile_pool(name="ps_sc", bufs=2, space="PSUM") as ps_sc,
            tc.tile_pool(name="ps_o", bufs=1, space="PSUM") as ps_o,
            tc.tile_pool(name="ps_d", bufs=1, space="PSUM") as ps_d,
        ):
            pools = (singles, prb_p, ps_proj, ps_sc, ps_o, ps_d)
            for _ in range(repeat):
                _emit_body(nc, tc, pools, dram)

    nc.compile()
    _CACHE[key] = nc
    return nc


def _prep_in_maps(X, padding_mask, W_q, W_k, W_v):
    X = np.asarray(X, dtype=np.float32)
    padding_mask = np.asarray(padding_mask, dtype=np.float32)

    def wprep(W):
        # [H, E] -> [E, H] -> [128(ei), EO, H] with ei innermost of E
        return np.asarray(W, dtype=np.float32).T.reshape(EO, 128, H).transpose(1, 0, 2)

    # [128, EO, 3, H]
    w3 = np.ascontiguousarray(
        np.stack([wprep(W_q), wprep(W_k), wprep(W_v)], axis=2)
    ).astype(ml_dtypes.bfloat16)
    ident = np.eye(128, dtype=np.float32)
    triA = -400.0 * np.triu(np.ones((128, 128), dtype=np.float32), 1)
    ones = np.ones((128, 128), dtype=np.float32)
    consts = np.ascontiguousarray(np.stack([ident, triA, ones], axis=1)).astype(
        ml_dtypes.bfloat16
    )  # [128, 3, 128]
    in_maps = []
    for b in range(B):
        Xm = X[b] * padding_mask[b][:, None]  # exact fp32 mask, then quantize
        in_maps.append(
            {
                "xt": np.ascontiguousarray(
                    # [S, E] -> [E, S] -> [128(ei), NJB, EO, 512]
                    Xm.T.reshape(EO, 128, NJB, 512).transpose(1, 2, 0, 3)
                ).astype(ml_dtypes.bfloat16),
                "w3": w3,
                "consts": consts,
            }
        )
    return in_maps


def _finish(res):
    # device wrote outT [128(h), S] bf16 and den [1, S]; out[q, h] = outT.T / den
    return (res["outT"].astype(np.float32).T / res["den"][0][:, None]).astype(
        np.float32
    )


def kernel(X, padding_mask, W_q, W_k, W_v):
    from concourse import bass2jax

    nc = _build(repeat=1)
    in_maps = _prep_in_maps(X, padding_mask, W_q, W_k, W_v)
    results = bass2jax.run_bass_via_pjrt(nc, in_maps, n_cores=B)
    return np.stack([_finish(results[b]) for b in range(B)], axis=0)


# revision 10
# speedup vs baseline: 1.2965x; 1.0679x over previous
"""Causal single-head attention on 8 Trainium2 NeuronCores.

Problem: B=8, S=2048, E=768, HEAD=128, fp32.
  Xm = X * padding_mask[:, :, None]
  q/k/v = Xm @ W_{q,k,v}.T          [B, S, H]
  scores = (q @ k.T) / sqrt(H)  (causal)
  out = softmax(scores) @ v          [B, S, H]

Sharding: pure data-parallel over batch - core b computes batch b; the
tiny projection weights are replicated to every core.

v4 design notes (on top of the bf16 v2 baseline):
  - Startup: inputs stream in ordered globally by first use, with the
    critical sequence (w3[eo01], xt0 in 3 eo-pair chunks, then xt1
    chunks + the fp8 copies) on the sync queue, which serves ~1.5us
    after kernel start at ~250GB/s. Per-chunk completion semaphores let
    each projection matmul wait only on its own chunk. xt2/xt3 bulk is
    issued mid-attention from the scalar engine's program (engine-order
    staging) so it cannot steal startup bandwidth.
  - Projection matmuls for block b+1 are interleaved as small work
    items into the attention pair-pipeline of block b, filling the PE
    bubbles that previously appeared while waiting on ACT exp. Block
    3's projection is split: qT/kT items run inside attn(2), vT +
    transpose items inside attn(3)'s early pairs (legal: pair g only
    touches k-tiles 2g,2g+1, so block-3 k/v tiles are needed only from
    pair 6 on).
  - q/k projections for blocks 2-3 run as fp8(e4m3) DoubleRow matmuls
    (2 contraction elements per cycle, halving their PE time). X and W
    are pre-scaled (x4 / x64) on the host so W escapes the e4m3
    subnormal range; the x65536 score scale folds into the exp scale
    and a second (scaled) causal-mask constant. Host-simulated rel err
    is unchanged (4.8e-3) because the max-error rows live in blocks
    0-1, which stay bf16.
  - Per-block qT/kT/vT/v SBUF tiles so interleaved projection writes
    can never alias attention reads of earlier blocks.
  - outT is drained in bf16 (host divides in fp32); blocks 0-2 go out
    as single 1KB-line transfers on the otherwise-idle gpsimd queue,
    block 3 in two halves on scalar/sync as soon as each half's
    accumulation completes. den goes out as one 8KB DMA at the end.
"""

import math
import sys

import numpy as np

sys.path.insert(0, "/opt/trn_rl_repo")

import ml_dtypes

B, S, E, H = 8, 2048, 768, 128
EO = E // 128          # 6 e-chunks
NJB = S // 512         # 4 q-blocks of 512
SCALE = float(1.0 / math.sqrt(H))
XS, WS = 4.0, 64.0     # fp8 pre-scales for X and W_q/W_k
F8 = (2, 3)            # q-blocks whose q/k projections run in fp8

_CACHE = {}


def _emit_body(nc, tc, pools, dram):
    import concourse.bass as bass  # noqa: F401
    from concourse import mybir

    f32 = mybir.dt.float32
    bf16 = mybir.dt.bfloat16
    fp8 = mybir.dt.float8e4
    DR = mybir.MatmulPerfMode.DoubleRow
    Exp = mybir.ActivationFunctionType.Exp

    singles, prb_p, ps_proj, ps_sc, ps_o, ps_d = pools
    (xt_d, xt8_d, w3_d, w8_d, consts_d, outT_d, den_d) = dram

    sb = _CACHE["sb"]
    if not sb:
        for jb in range(NJB):
            sb[f"xt{jb}"] = singles.tile(
                [128, EO, 512], bf16, tag=f"xt{jb}", name=f"xt{jb}"
            )
            for nm in ("qT", "kT", "vT", "v"):
                sb[f"{nm}{jb}"] = singles.tile(
                    [128, 512], bf16, tag=f"{nm}{jb}", name=f"{nm}{jb}"
                )
            sb[f"outF{jb}"] = singles.tile(
                [128, 512], bf16, tag=f"outF{jb}", name=f"outF{jb}"
            )
        for jb in F8:
            sb[f"xt8_{jb}"] = singles.tile(
                [128, 3, 2, 512], fp8, tag=f"xt8_{jb}", name=f"xt8_{jb}"
            )
        sb["w8"] = singles.tile([128, 3, 2, 2, H], fp8, tag="w8", name="w8")
        sb["w3"] = singles.tile([128, EO, 3, H], bf16, tag="w3", name="w3")
        sb["consts"] = singles.tile([128, 4, 128], bf16, tag="consts", name="consts")
        sb["denF"] = singles.tile([1, S], f32, tag="denF", name="denF")
        sb["warm"] = singles.tile([128, 512], bf16, tag="warm", name="warm")

    xt_ap = xt_d.ap()
    xt8_ap = xt8_d.ap()
    w3_ap = w3_d.ap()
    outT_ap = outT_d.ap()
    den_ap = den_d.ap()

    # ---- prologue loads --------------------------------------------------
    # sync queue: the startup-critical sequence, ordered by first use.
    # scalar joins ~2us later with the rest of w3 + consts; its bulk
    # (xt2/xt3) is staged from inside attn(0)/attn(1) instead. gpsimd
    # stays free for output drains.
    nc.gpsimd.memset(sb["warm"], 0.125)
    nc.sync.dma_start(out=sb["w3"][:, 0:2], in_=w3_ap[:, 0:2])
    for p in range(3):
        nc.sync.dma_start(
            out=sb["xt0"][:, 2 * p : 2 * p + 2], in_=xt_ap[:, 0, 2 * p : 2 * p + 2]
        )
    for p in range(3):
        nc.sync.dma_start(
            out=sb["xt1"][:, 2 * p : 2 * p + 2], in_=xt_ap[:, 1, 2 * p : 2 * p + 2]
        )
    nc.sync.dma_start(out=sb["w8"], in_=w8_d.ap())
    for i, jb in enumerate(F8):
        nc.sync.dma_start(out=sb[f"xt8_{jb}"], in_=xt8_ap[:, i])
    nc.scalar.dma_start(out=sb["w3"][:, 2:4], in_=w3_ap[:, 2:4])
    nc.scalar.dma_start(out=sb["w3"][:, 4:6], in_=w3_ap[:, 4:6])
    nc.scalar.dma_start(out=sb["consts"], in_=consts_d.ap())

    ident = sb["consts"][:, 0, :]
    triA = sb["consts"][:, 1, :]       # -400 (bf16-scale blocks)
    triA8 = sb["consts"][:, 2, :]      # -400 * (XS*WS)^2 (fp8-scale blocks)
    ones1 = sb["consts"][:, 3, 0:1]

    # Short PE warmup bridging the ~2us until the first chunks land;
    # primes the HAM activity window so real matmuls warm up sooner.
    ps_warm = ps_proj.tile([128, 512], f32, tag="proj", name="ps_warm")
    for _ in range(5):
        nc.tensor.matmul(
            ps_warm, lhsT=sb["warm"][:, 0:128], rhs=sb["warm"], start=True, stop=True
        )

    # ---- projection work items ------------------------------------------
    # proj(b) = 11 small PE items: 3 per weight (chunk-paced; the last
    # one adds the PSUM->SBUF copy) + 2 transpose items for v. For F8
    # blocks the q/k items are single fp8 DoubleRow matmuls over an
    # eo-pair (256-wide contraction at 2 elems/cycle).
    def proj_items(b, parts=(0, 1, 2)):
        items = []
        cell = {}
        use8 = b in F8
        x8 = sb[f"xt8_{b}"] if use8 else None

        def mm_item(wi, p, nm, b=b):
            def run():
                if p == 0:
                    cell[wi] = ps_proj.tile(
                        [128, 512], f32, tag="proj", name=f"ps_{nm}{b}"
                    )
                ps = cell[wi]
                if use8 and wi < 2:
                    nc.tensor.matmul(
                        ps,
                        lhsT=sb["w8"][:, p, :, wi, :],
                        rhs=x8[:, p, :, :],
                        start=(p == 0),
                        stop=(p == 2),
                        perf_mode=DR,
                    )
                else:
                    for eo in (2 * p, 2 * p + 1):
                        nc.tensor.matmul(
                            ps,
                            lhsT=sb["w3"][:, eo, wi, :],
                            rhs=sb[f"xt{b}"][:, eo, :],
                            start=(eo == 0),
                            stop=(eo == EO - 1),
                        )
                if p == 2:
                    nc.vector.tensor_copy(sb[f"{nm}{b}"], ps)

            return run

        def tr_item(half, b=b):
            def run():
                if half == 0:
                    cell["psv"] = ps_proj.tile(
                        [128, 512], bf16, tag="proj", name=f"psv{b}"
                    )
                psv = cell["psv"]
                for c in (2 * half, 2 * half + 1):
                    nc.tensor.transpose(
                        psv[:, 128 * c : 128 * (c + 1)],
                        sb[f"vT{b}"][:, 128 * c : 128 * (c + 1)],
                        ident,
                    )
                if half == 1:
                    nc.vector.tensor_copy(sb[f"v{b}"], psv)

            return run

        for wi, nm in ((0, "qT"), (1, "kT"), (2, "vT")):
            if wi not in parts:
                continue
            for p in range(3):
                items.append(mm_item(wi, p, nm))
        if 2 in parts:
            items.append(tr_item(0))
            items.append(tr_item(1))
        return items

    # ---- attention for q-block b, interleaving `items` into the pairs ----
    def attn(b, items, stage=None):
        nkt = 4 * (b + 1)          # causal: k tiles 0 .. 4b+3
        npr = nkt // 2
        pso = ps_o.tile([128, 512], f32, tag="o", name=f"pso_{b}")
        psd = ps_d.tile([1, 512], f32, tag="d", name=f"psd_{b}")
        qT = sb[f"qT{b}"]
        sq = 1.0 / (XS * WS) if b in F8 else 1.0

        def off_of(i):
            return 128 * (i - 4 * b) if i >= 4 * b else 0

        def kt_of(i):
            return sb[f"kT{i // 4}"][:, 128 * (i % 4) : 128 * (i % 4 + 1)]

        def v_of(i):
            return sb[f"v{i // 4}"][:, 128 * (i % 4) : 128 * (i % 4 + 1)]

        def emit_scores(g):
            kb = (2 * g) // 4      # pairs never straddle a k-block boundary
            sk = 1.0 / (XS * WS) if kb in F8 else 1.0
            pssc = ps_sc.tile([128, 2, 512], f32, tag="sc", name=f"sc_{b}_{g}")
            for t in range(2):
                i = 2 * g + t
                diag = i >= 4 * b
                off = off_of(i)
                nc.tensor.matmul(
                    pssc[:, t, off:],
                    lhsT=kt_of(i),
                    rhs=qT[:, off:],
                    start=True,
                    stop=not diag,
                )
                if diag:  # add -400*(scale) strictly-upper triangle pre-exp
                    nc.tensor.matmul(
                        pssc[:, t, off : off + 128],
                        lhsT=(triA8 if b in F8 else triA),
                        rhs=ident,
                        start=False,
                        stop=True,
                    )
            moff = off_of(2 * g)
            prb = prb_p.tile([128, 2, 512], bf16, tag="pr", name=f"prb_{b}_{g}")
            nc.scalar.activation(
                prb[:, :, moff:], pssc[:, :, moff:], Exp, scale=SCALE * sq * sk
            )
            return (g, prb)

        def emit_outden(pend, last):
            g, pprb = pend
            for t in range(2):
                i = 2 * g + t
                off = off_of(i)
                nc.tensor.matmul(
                    pso[:, off:],
                    lhsT=v_of(i),
                    rhs=pprb[:, t, off:],
                    start=(i == 0),
                    stop=last and t == 1,
                )
            for t in range(2):
                i = 2 * g + t
                off = off_of(i)
                nc.tensor.matmul(
                    psd[:, off:],
                    lhsT=ones1,
                    rhs=pprb[:, t, off:],
                    start=(i == 0),
                    stop=last and t == 1,
                )
            # cols [0:256] final once the off=128 diag tile has run: drain
            # early so the tail copy+DMA overlaps the last pair. Only the
            # last block DMAs its halves separately (tail latency); the
            # others go out as one 1KB-line transfer on the idle gpsimd
            # queue after the tail copy.
            if g == npr - 2:
                nc.vector.tensor_copy(sb[f"outF{b}"][:, 0:256], pso[:, 0:256])
                if b == NJB - 1:
                    nc.scalar.dma_start(
                        out=outT_ap[:, 512 * b : 512 * b + 256],
                        in_=sb[f"outF{b}"][:, 0:256],
                    )

        # spread items over the early pairs (all before pair npr-2 when
        # possible, so block-3's own k/v items land before they're read).
        spread = max(1, min(npr - 1, 6))
        pipe = []
        for g in range(npr):
            pipe.append(emit_scores(g))
            if g == npr - 1 and stage is not None:
                stage()  # engine-order staged bulk prefetch (scalar queue)
            if g < spread and items:
                budget = (len(items) + (spread - g) - 1) // (spread - g)
                for _ in range(budget):
                    if items:
                        items.pop(0)()
            if len(pipe) > 1:
                emit_outden(pipe.pop(0), last=False)
        while items:  # leftovers (small blocks)
            items.pop(0)()
        while pipe:
            p = pipe.pop(0)
            emit_outden(p, last=not pipe)

        # tail drain: cols [256:512] + this block's denominators
        nc.vector.tensor_copy(sb[f"outF{b}"][:, 256:512], pso[:, 256:])
        if b == NJB - 1:
            nc.sync.dma_start(
                out=outT_ap[:, 512 * b + 256 : 512 * (b + 1)],
                in_=sb[f"outF{b}"][:, 256:512],
            )
        else:
            nc.gpsimd.dma_start(
                out=outT_ap[:, 512 * b : 512 * (b + 1)], in_=sb[f"outF{b}"]
            )
        nc.vector.tensor_copy(sb["denF"][0:1, 512 * b : 512 * (b + 1)], psd)

    # ---- schedule --------------------------------------------------------
    for it in proj_items(0):
        it()
    attn(0, proj_items(1),
         stage=lambda: nc.scalar.dma_start(out=sb["xt2"], in_=xt_ap[:, 2]))
    attn(1, proj_items(2),
         stage=lambda: nc.scalar.dma_start(out=sb["xt3"], in_=xt_ap[:, 3]))
    attn(2, proj_items(3, parts=(0, 1)))       # qT/kT of block 3
    attn(3, proj_items(3, parts=(2,)))         # vT + transposes of block 3
    nc.scalar.dma_start(out=den_ap, in_=sb["denF"])


def _build(repeat=1):
    key = ("nc", repeat)
    if key in _CACHE:
        return _CACHE[key]

    import concourse.tile as tile
    from concourse import bacc, mybir

    f32 = mybir.dt.float32
    bf16 = mybir.dt.bfloat16
    fp8 = mybir.dt.float8e4
    nc = bacc.Bacc("TRN2", target_bir_lowering=False, debug=False)

    xt_d = nc.dram_tensor("xt", [128, NJB, EO, 512], bf16, kind="ExternalInput")
    xt8_d = nc.dram_tensor(
        "xt8", [128, len(F8), 3, 2, 512], fp8, kind="ExternalInput"
    )
    w3_d = nc.dram_tensor("w3", [128, EO, 3, H], bf16, kind="ExternalInput")
    w8_d = nc.dram_tensor("w8", [128, 3, 2, 2, H], fp8, kind="ExternalInput")
    consts_d = nc.dram_tensor("consts", [128, 4, 128], bf16, kind="ExternalInput")
    outT_d = nc.dram_tensor("outT", [128, S], bf16, kind="ExternalOutput")
    den_d = nc.dram_tensor("den", [1, S], f32, kind="ExternalOutput")
    dram = (xt_d, xt8_d, w3_d, w8_d, consts_d, outT_d, den_d)

    _CACHE["sb"] = {}
    with tile.TileContext(nc) as tc:
        with (
            tc.tile_pool(name="singles", bufs=1) as singles,
            tc.tile_pool(name="probs", bufs=6) as prb_p,
            tc.tile_pool(name="ps_proj", bufs=2, space="PSUM") as ps_proj,
            tc.tile_pool(name="ps_sc", bufs=2, space="PSUM") as ps_sc,
            tc.tile_pool(name="ps_o", bufs=1, space="PSUM") as ps_o,
            tc.tile_pool(name="ps_d", bufs=1, space="PSUM") as ps_d,
        ):
            pools = (singles, prb_p, ps_proj, ps_sc, ps_o, ps_d)
            for _ in range(repeat):
                _emit_body(nc, tc, pools, dram)

    nc.compile()
    _CACHE[key] = nc
    return nc


def _prep_in_maps(X, padding_mask, W_q, W_k, W_v):
    e4 = ml_dtypes.float8_e4m3
    X = np.asarray(X, dtype=np.float32)
    padding_mask = np.asarray(padding_mask, dtype=np.float32)

    def wprep(W):
        # [H, E] -> [E, H] -> [128(ei), EO, H] with ei innermost of E
        return np.asarray(W, dtype=np.float32).T.reshape(EO, 128, H).transpose(1, 0, 2)

    # [128, EO, 3, H]
    w3 = np.ascontiguousarray(
        np.stack([wprep(W_q), wprep(W_k), wprep(W_v)], axis=2)
    ).astype(ml_dtypes.bfloat16)

    def w8prep(W):
        # [H, E] -> [E, H] -> (3, 2, 128, H) -> [128(ki), 3(c), 2(ko), H]
        a = (np.asarray(W, dtype=np.float32).T * WS).reshape(3, 2, 128, H)
        return a.transpose(2, 0, 1, 3)

    # [128, 3, 2, 2(wi=q,k), H]
    w8 = np.ascontiguousarray(
        np.stack([w8prep(W_q), w8prep(W_k)], axis=3)
    ).astype(e4)

    ident = np.eye(128, dtype=np.float32)
    tri = np.triu(np.ones((128, 128), dtype=np.float32), 1)
    ones = np.ones((128, 128), dtype=np.float32)
    consts = np.ascontiguousarray(
        np.stack(
            [ident, -400.0 * tri, -400.0 * (XS * WS) ** 2 * tri, ones], axis=1
        )
    ).astype(ml_dtypes.bfloat16)  # [128, 4, 128]

    in_maps = []
    for b in range(B):
        Xm = X[b] * padding_mask[b][:, None]  # exact fp32 mask, then quantize
        # [S, E] -> [E, S] -> (3, 2, 128, NJB, 512) -> [128, NJB, 3, 2, 512]
        x8full = (
            (Xm.T * XS)
            .reshape(3, 2, 128, NJB, 512)
            .transpose(2, 3, 0, 1, 4)
        )
        in_maps.append(
            {
                "xt": np.ascontiguousarray(
                    # [S, E] -> [E, S] -> [128(ei), NJB, EO, 512]
                    Xm.T.reshape(EO, 128, NJB, 512).transpose(1, 2, 0, 3)
                ).astype(ml_dtypes.bfloat16),
                "xt8": np.ascontiguousarray(x8full[:, list(F8)]).astype(e4),
                "w3": w3,
                "w8": w8,
                "consts": consts,
            }
        )
    return in_maps


def _finish(res):
    # device wrote outT [128(h), S] bf16 and den [1, S]; out[q, h] = outT.T / den
    return (res["outT"].astype(np.float32).T / res["den"][0][:, None]).astype(
        np.float32
    )


def kernel(X, padding_mask, W_q, W_k, W_v):
    from concourse import bass2jax

    nc = _build(repeat=1)
    in_maps = _prep_in_maps(X, padding_mask, W_q, W_k, W_v)
    results = bass2jax.run_bass_via_pjrt(nc, in_maps, n_cores=B)
    return np.stack([_finish(results[b]) for b in range(B)], axis=0)
